# revision 1
# baseline (speedup 1.0000x reference)
"""AtomAttentionPairBias Trainium2 kernel (8 NeuronCores, SPMD, no collectives).

Local atom attention (AF3-style): 2048 queries in 32-query blocks, each block
attending a 128-wide key window.  Core c owns 256 queries (8 blocks) plus a
384-row key/value halo; the (N,N,Cz) pair tensor is sliced to the attended
band on the host (2MB/core instead of 256MB).
"""

import functools
import sys

import numpy as np

sys.path.insert(0, "/opt/trn_rl_repo")

import ml_dtypes  # noqa: E402

import concourse.bass as bass  # noqa: E402
import concourse.tile as tile  # noqa: E402
from concourse import bacc, mybir  # noqa: E402
from concourse.bass_utils import run_bass_kernel_spmd  # noqa: E402

BF16 = mybir.dt.bfloat16
F32 = mybir.dt.float32

N, C_IN, C_Z, H, C = 2048, 128, 16, 4, 32
QB, WL, WR = 32, 48, 80
NCORES = 8
RQ = N // NCORES          # 256 query rows per core
NB = RQ // QB             # 8 blocks per core
W = WL + WR               # 128-wide key window
RK = 384                  # padded key halo rows per core (352 used)
EPS = 1e-5
NEG = -1e9


def _build():
    nc = bacc.Bacc("TRN2", detect_race_conditions=False)

    def din(name, shape, dt=BF16):
        return nc.declare_dram_parameter(name, list(shape), dt, isOutput=False)

    # per-core activations (transposed: [channel, row]): xq|sq|xk|sk
    xs = din("xs", (C_IN, 2 * RQ + 2 * RK))
    zt = din("zt", (128, QB * W))                 # [g*16+cz, i*128+j] per block g
    # weights / constants (shared across cores), coalesced into 3 loads:
    # wcat cols: 12x128 weights | wstat 72 | ssel 16 | ident 128
    wcat = din("wcat", (C_IN, 12 * 128 + 72 + 16 + 128))
    small4 = din("small4", (4, 128 + 2 * W))      # e4 | edge[(quad,j)]
    sel2 = din("sel2", (2, 2, C_IN))              # K=2 row-selector lhsT
    biasv = din("biasv", (C_IN, 4), F32)          # bgq | bgk | bq | bgs
    out_d = nc.declare_dram_parameter("out", [C_IN, RQ], F32, isOutput=True)

    AF = mybir.ActivationFunctionType
    ALU = mybir.AluOpType

    with tile.TileContext(nc) as tc:
        with (
            tc.tile_pool(name="const", bufs=1) as cp,
            tc.tile_pool(name="act", bufs=1) as ap,
            tc.tile_pool(name="ps1", bufs=3, space="PSUM") as ps1,
            tc.tile_pool(name="ps_z", bufs=2, space="PSUM") as ps_z,
            tc.tile_pool(name="ps_sc", bufs=2, space="PSUM") as ps_sc,
            tc.tile_pool(name="ps_at", bufs=1, space="PSUM") as ps_at,
        ):
            def load(dram, tag, dt=None):
                t = cp.tile(list(dram.shape), dt or dram.dtype, tag=tag)
                nc.sync.dma_start(out=t[:], in_=dram[:])
                return t

            zeros_t = cp.tile([128, 1], F32, tag="zeros")
            nc.vector.memset(zeros_t[:], 0.0)
            eps_t = cp.tile([128, 1], F32, tag="eps")
            nc.vector.memset(eps_t[:], EPS)
            nc.const_aps.aps[(F32, 0.0)] = zeros_t[:]
            nc.const_aps.aps[(F32, EPS)] = eps_t[:]

            t_wcat = load(wcat, "wcat")
            # big z: two DMAs so chunk compute can start at half load
            t_zt = cp.tile(list(zt.shape), zt.dtype, tag="zt")
            half = QB * W // 2
            nc.sync.dma_start(out=t_zt[:, 0:half], in_=zt[:, 0:half])
            nc.sync.dma_start(out=t_zt[:, half:], in_=zt[:, half:])
            t_xs = load(xs, "xs")
            t_xq = t_xs[:, 0:RQ]
            t_sq = t_xs[:, RQ:2 * RQ]
            t_xk = t_xs[:, 2 * RQ:2 * RQ + RK]
            t_sk = t_xs[:, 2 * RQ + RK:]
            t_sel2 = load(sel2, "sel2")
            t_small4 = load(small4, "small4")
            t_biasv = load(biasv, "biasv", F32)
            wslc = lambda i: t_wcat[:, i * 128:(i + 1) * 128]
            (t_wgq, t_wgk, t_wq_m, t_wq_s, t_wg_m, t_wg_s,
             t_wk_m, t_wk_s, t_wv_m, t_wv_s, t_wo, t_wgs) = [
                wslc(i) for i in range(12)]
            t_wstat = t_wcat[:, 1536:1608]
            t_ssel = t_wcat[:, 1608:1624].rearrange("p (r c) -> p r c", c=4)
            t_id = t_wcat[:, 1624:1752]
            t_e4 = t_small4[:, 0:128]
            t_edge = t_small4[:, 128:].rearrange("p (q j) -> p q j", j=W)
            t_bgq = t_biasv[:, 0:1]; t_bgk = t_biasv[:, 1:2]
            t_bq = t_biasv[:, 2:3]; t_bgs = t_biasv[:, 3:4]

            # ---- LayerNorm of x/s (both branches), transposed layout ----
            # squares on ACT
            x2q = ap.tile([128, RQ], BF16, tag="x2q"); nc.scalar.square(x2q[:], t_xq)
            s2q = ap.tile([128, RQ], BF16, tag="s2q"); nc.scalar.square(s2q[:], t_sq)
            x2k = ap.tile([128, RK], BF16, tag="x2k"); nc.scalar.square(x2k[:], t_xk)
            s2k = ap.tile([128, RK], BF16, tag="s2k"); nc.scalar.square(s2k[:], t_sk)

            # stats via col-hot ones matmuls: rows (m_x, m_s) / (e_x, e_s)
            stm_q = ps1.tile([2, RQ], F32, tag="ps1")
            ste_q = ps1.tile([2, RQ], F32, tag="ps1")
            stm_k = ps1.tile([2, RK], F32, tag="ps1")
            ste_k = ps1.tile([2, RK], F32, tag="ps1")
            for st, rhs0, rhs1 in ((stm_q, t_xq, t_sq), (ste_q, x2q, s2q),
                                   (stm_k, t_xk, t_sk), (ste_k, x2k, s2k)):
                nc.tensor.matmul(st[:, :], t_ssel[:, 0, 0:2], rhs0[:],
                                 start=True, stop=False)
                nc.tensor.matmul(st[:, :], t_ssel[:, 1, 0:2], rhs1[:],
                                 start=False, stop=True)

            # var = E[x^2] - m^2 ; rs = exp(-0.5*log(var+eps)) ; m -> sbuf bf16
            def ln_stats(stm, ste, R, tag):
                mcp = ap.tile([2, R], F32, tag=f"mcp_{tag}")
                nc.vector.tensor_copy(mcp[:], stm[:, :])
                msq = ap.tile([2, R], F32, tag=f"msq_{tag}")
                nc.vector.tensor_mul(msq[:], mcp[:], mcp[:])
                var = ap.tile([2, R], F32, tag=f"var_{tag}")
                nc.vector.tensor_tensor(var[:], ste[:, :], msq[:], op=ALU.subtract)
                lg = ap.tile([2, R], F32, tag=f"lg_{tag}")
                nc.scalar.activation(lg[:], var[:], AF.Ln, bias=EPS)
                rs = ap.tile([2, R], BF16, tag=f"rs_{tag}")
                nc.scalar.activation(rs[:], lg[:], AF.Exp, scale=-0.5)
                m = ap.tile([2, R], BF16, tag=f"m_{tag}")
                nc.vector.tensor_copy(m[:], mcp[:])  # bf16 for matmul rhs
                return m, rs

            m_q, rs_q = ln_stats(stm_q, ste_q, RQ, "q")
            m_k, rs_k = ln_stats(stm_k, ste_k, RK, "k")

            # broadcast + apply:  xn = (x - m) * rs  (row-select K=2 matmuls)
            def ln_apply(xT, m2, rs2, row, R, tag):
                mbc = ps1.tile([128, R], F32, tag="ps1")
                nc.tensor.matmul(mbc[:], t_sel2[:, row, :], m2[:, :R])
                rbc = ps1.tile([128, R], F32, tag="ps1")
                nc.tensor.matmul(rbc[:], t_sel2[:, row, :], rs2[:, :R])
                cen = ap.tile([128, R], BF16, tag=f"cen_{tag}")
                nc.vector.tensor_tensor(cen[:], xT[:], mbc[:], op=ALU.subtract)
                xn = ap.tile([128, R], BF16, tag=f"xn_{tag}")
                nc.vector.tensor_mul(xn[:], cen[:], rbc[:])
                return xn

            xnq = ln_apply(t_xq, m_q, rs_q, 0, RQ, "xq")
            snq = ln_apply(t_sq, m_q, rs_q, 1, RQ, "sq")
            xnk = ln_apply(t_xk, m_k, rs_k, 0, RK, "xk")
            snk = ln_apply(t_sk, m_k, rs_k, 1, RK, "sk")

            # ---- adaLN gates:  M = sigmoid(W_gate @ sn + b) * xn ----
            def gate(wg, sn, bg, xn, R, tag):
                g_ps = ps1.tile([128, R], F32, tag="ps1")
                nc.tensor.matmul(g_ps[:], wg, sn[:])
                sg = ap.tile([128, R], BF16, tag=f"sg_{tag}")
                nc.scalar.activation(sg[:], g_ps[:], AF.Sigmoid, bias=bg)
                mm = ap.tile([128, R], BF16, tag=f"M_{tag}")
                nc.vector.tensor_mul(mm[:], sg[:], xn[:])
                return mm

            Mq = gate(t_wgq, snq, t_bgq, xnq, RQ, "q")
            Mk = gate(t_wgk, snk, t_bgk, xnk, RK, "k")

            # ---- projections (skip path folded into _s weights) ----
            qT_ps = ps1.tile([128, RQ], F32, tag="ps1")
            nc.tensor.matmul(qT_ps[:], t_wq_m, Mq[:], start=True, stop=False)
            nc.tensor.matmul(qT_ps[:], t_wq_s, snq[:], start=False, stop=True)
            qT = ap.tile([128, RQ], BF16, tag="qT")
            nc.vector.tensor_scalar_add(qT[:], qT_ps[:], t_bq)

            gT_ps = ps1.tile([128, RQ], F32, tag="ps1")
            nc.tensor.matmul(gT_ps[:], t_wg_m, Mq[:], start=True, stop=False)
            nc.tensor.matmul(gT_ps[:], t_wg_s, snq[:], start=False, stop=True)
            sig_g = ap.tile([128, RQ], BF16, tag="sig_g")
            nc.scalar.activation(sig_g[:], gT_ps[:], AF.Sigmoid)

            kT_ps = ps1.tile([128, RK], F32, tag="ps1")
            nc.tensor.matmul(kT_ps[:], t_wk_m, Mk[:], start=True, stop=False)
            nc.tensor.matmul(kT_ps[:], t_wk_s, snk[:], start=False, stop=True)
            kT = ap.tile([128, RK], BF16, tag="kT")
            nc.vector.tensor_copy(kT[:], kT_ps[:])

            # v computed directly in skewed layout: slot b = window rows
            # [32b, 32b+128) of natural v (lhsT free-dim slice at offset 32b)
            v_skew = ap.tile([128, NB, 128], BF16, tag="v_skew")
            for b in range(NB):
                v_ps = ps1.tile([128, 128], F32, tag="ps1")
                sl = bass.ds(QB * b, 128)
                nc.tensor.matmul(v_ps[:], Mk[:, sl], t_wv_m, start=True, stop=False)
                nc.tensor.matmul(v_ps[:], snk[:, sl], t_wv_s, start=False, stop=True)
                if b % 2 == 0:
                    nc.vector.tensor_copy(v_skew[:, b, :], v_ps[:])
                else:
                    nc.scalar.copy(v_skew[:, b, :], v_ps[:])

            # ---- pair-bias: blockdiag LN-projection of z ----
            NCH = 8  # chunks of 512 columns
            zw_sb = ap.tile([48, NB * QB * W // 8], BF16, tag="zw_sb")  # [48, 4096]
            for t in range(NCH):
                sl = bass.ts(t, 512)
                z2 = ap.tile([128, 512], BF16, tag="z2")
                if t % 2 == 0:
                    nc.vector.tensor_mul(z2[:], t_zt[:, sl], t_zt[:, sl])
                else:
                    nc.scalar.square(z2[:], t_zt[:, sl])
                zc = ps_z.tile([64, 512], F32, tag="zc")
                # z^2 pass first (rows 32-63, zeros over m-rows); z pass then
                # overwrites rows 0-39.  Leaves [48,512]: proj | mean | E[z^2].
                nc.tensor.matmul(zc[32:64, :], t_wstat[:, 40:72], z2[:],
                                 tile_position=(0, 32))
                nc.tensor.matmul(zc[0:40, :], t_wstat[:, 0:40], t_zt[:, sl])
                if t % 2 == 0:
                    nc.scalar.copy(zw_sb[:, sl], zc[0:48, :])
                else:
                    nc.vector.tensor_copy(zw_sb[:, sl], zc[0:48, :])

            # reshape zW into per-(quad,head) score-layout tiles + dense stats
            T_m = ap.tile([128, 2 * W], BF16, tag="T_m")
            T_e = ap.tile([128, 2 * W], BF16, tag="T_e")
            for Q in range(2):
                for row0, dst in ((32, T_m), (40, T_e)):
                    r = row0 + 4 * Q
                    nc.sync.dma_start(
                        out=dst[:, bass.ts(Q, W)],
                        in_=zw_sb[r:r + 4, :].rearrange("b (i j) -> b i j", j=W))
            bias_t = []
            for t1 in range(NB):
                Q, h = t1 // 4, t1 % 4
                bt = ap.tile([128, W], BF16, tag=f"bias_{t1}")
                r = h * 8 + 4 * Q
                nc.sync.dma_start(
                    out=bt[:],
                    in_=zw_sb[r:r + 4, :].rearrange("b (i j) -> b i j", j=W))
                bias_t.append(bt)
            msqd = ap.tile([128, 2 * W], BF16, tag="msqd")
            nc.vector.tensor_mul(msqd[:], T_m[:], T_m[:])
            vard = ap.tile([128, 2 * W], F32, tag="vard")
            nc.vector.tensor_tensor(vard[:], T_e[:], msqd[:], op=ALU.subtract)
            lgd = ap.tile([128, 2 * W], F32, tag="lgd")
            nc.scalar.activation(lgd[:], vard[:], AF.Ln, bias=EPS)
            T_rs = ap.tile([128, 2 * W], BF16, tag="T_rs")
            nc.scalar.activation(T_rs[:], lgd[:], AF.Exp, scale=-0.5)

            biasrs_t = []
            for t1 in range(NB):
                Q = t1 // 4
                brt = ap.tile([128, W], BF16, tag=f"biasrs_{t1}")
                nc.vector.tensor_mul(brt[:], bias_t[t1][:], T_rs[:, bass.ts(Q, W)])
                biasrs_t.append(brt)

            # ---- scores ----
            # per-(quad,head) score tiles: own PSUM bank => self-contained groups
            A_sb = ap.tile([128, NB, W], BF16, tag="A_sb")
            sums = ap.tile([128, NB], F32, tag="sums")
            for Q in range(2):
                for h in range(H):
                    t1 = Q * 4 + h
                    sc = ps_sc.tile([128, W], F32, tag="scores")
                    # edge mask first: covers all partitions, opens the group
                    nc.tensor.matmul(sc[:, :], t_e4, t_edge[:, Q, :],
                                     start=True, stop=False)
                    for b4 in range(4):
                        b = Q * 4 + b4
                        nc.tensor.matmul(
                            sc[b4 * 32:b4 * 32 + 32, :],
                            qT[h * 32:h * 32 + 32, bass.ts(b, QB)],
                            kT[h * 32:h * 32 + 32, QB * b:QB * b + W],
                            start=False, stop=False,
                            tile_position=(32 * h, 32 * b4))
                    nc.tensor.matmul(sc[:, :], t_id, biasrs_t[t1][:],
                                     start=False, stop=True)
                    # softmax (no max-subtraction; scores are O(1))
                    nc.scalar.activation(A_sb[:, t1, :], sc[:, :], AF.Exp,
                                         accum_out=sums[:, t1:t1 + 1])
            rec = ap.tile([128, NB], F32, tag="rec")
            nc.vector.reciprocal(rec[:], sums[:])
            An = ap.tile([128, NB, W], BF16, tag="An")
            recb = rec[:, :].rearrange("p (t o) -> p t o", o=1).to_broadcast((128, NB, W))
            nc.vector.tensor_mul(An[:], A_sb[:, :, :], recb)

            # ---- A^T then PV ----
            at_ps = ps_at.tile([128, NB, W], BF16, tag="at")
            for t1 in range(NB):
                nc.tensor.transpose(at_ps[:, t1, :], An[:, t1, :], t_id)
            At = ap.tile([128, NB, W], BF16, tag="At")
            nc.vector.tensor_copy(At[:], at_ps[:])

            ot_ps = ps1.tile([128, NB, QB], F32, tag="ps1")
            for b in range(NB):
                Q, b4 = b // 4, b % 4
                for h in range(H):
                    nc.tensor.matmul(
                        ot_ps[h * 32:h * 32 + 32, b, :],
                        v_skew[:, b, h * 32:h * 32 + 32],
                        At[:, Q * 4 + h, b4 * 32:b4 * 32 + 32],
                        tile_position=(0, 32 * h))

            ot_sb = ap.tile([128, RQ], BF16, tag="ot_sb")
            nc.vector.tensor_mul(ot_sb[:], ot_ps[:, :, :].rearrange("p a b -> p (a b)"),
                                 sig_g[:])

            # ---- output: sigmoid(Wgs@cond+bgs) * (Wo@o) ----
            fin_ps = ps1.tile([128, RQ], F32, tag="ps1")
            nc.tensor.matmul(fin_ps[:], t_wo, ot_sb[:])
            g2_ps = ps1.tile([128, RQ], F32, tag="ps1")
            nc.tensor.matmul(g2_ps[:], t_wgs, t_sq)
            sig2 = ap.tile([128, RQ], BF16, tag="sig2")
            nc.scalar.activation(sig2[:], g2_ps[:], AF.Sigmoid, bias=t_bgs)
            out_sb = ap.tile([128, RQ], F32, tag="out_sb")
            hf = RQ // 2
            nc.vector.tensor_mul(out_sb[:, 0:hf], fin_ps[:, 0:hf], sig2[:, 0:hf])
            nc.sync.dma_start(out=out_d[:, 0:hf], in_=out_sb[:, 0:hf])
            nc.vector.tensor_mul(out_sb[:, hf:], fin_ps[:, hf:], sig2[:, hf:])
            nc.sync.dma_start(out=out_d[:, hf:], in_=out_sb[:, hf:])

    nc.compile()
    return nc


@functools.lru_cache(maxsize=1)
def _built():
    return _build()


def _bf(a):
    return np.ascontiguousarray(a.astype(ml_dtypes.bfloat16))


def kernel(single_act, pair_act, single_cond, block_mask,
           lns_q, Wgate_q, bgate_q, Wskip_q,
           lns_k, Wgate_k, bgate_k, Wskip_k,
           lnz_w, Wq, bq, Wk, Wv, Wg, Wb, Wo, Wgs, bgs, **_ignored):
    single_act = np.asarray(single_act, np.float32)
    pair_act = np.asarray(pair_act, np.float32)
    single_cond = np.asarray(single_cond, np.float32)
    block_mask = np.asarray(block_mask)
    f = lambda a: np.asarray(a, np.float32)

    # ---- fold weights on host ----
    sc = 1.0 / np.sqrt(np.float32(C))
    wskq = f(lns_q)[:, None] * f(Wskip_q)
    wskk = f(lns_k)[:, None] * f(Wskip_k)
    w12 = [f(lns_q)[:, None] * f(Wgate_q), f(lns_k)[:, None] * f(Wgate_k),
           f(Wq) * sc, wskq @ f(Wq) * sc,
           f(Wg), wskq @ f(Wg),
           f(Wk), wskk @ f(Wk),
           f(Wv), wskk @ f(Wv),
           f(Wo), f(Wgs)]
    biasv = np.stack([f(bgate_q), f(bgate_k), f(bq) * sc, f(bgs)], 1)
    shared = {"biasv": np.ascontiguousarray(biasv, dtype=np.float32)}
    sel2 = np.zeros((2, 2, C_IN), np.float32)
    sel2[0, 0, :] = 1.0
    sel2[1, 1, :] = 1.0
    shared["sel2"] = _bf(sel2)
    # z projection stationary: cols 0-31 centered proj (h*8+g), 32-39 mean, 40-47 E[z^2]
    Wp = f(lnz_w)[:, None] * f(Wb)                       # [16, 4]
    Wpp = Wp - np.ones((C_Z, 1), np.float32) @ Wp.sum(0, keepdims=True) / C_Z
    wstat = np.zeros((128, 72), np.float32)
    for g in range(8):
        for h in range(H):
            wstat[g * 16:(g + 1) * 16, h * 8 + g] = Wpp[:, h]
        wstat[g * 16:(g + 1) * 16, 32 + g] = 1.0 / C_Z       # mean (z pass)
        wstat[g * 16:(g + 1) * 16, 48 + g] = 1.0 / C_Z       # E[z^2] (z^2 pass)
    ssel = np.zeros((C_IN, 16), np.float32)
    for r in range(4):
        ssel[:, r * 4 + r] = 1.0 / 128.0
    shared["wcat"] = _bf(np.concatenate(
        w12 + [wstat, ssel, np.eye(128, dtype=np.float32)], axis=1))
    e4 = np.zeros((4, 128), np.float32)
    for b4 in range(4):
        e4[b4, b4 * 32:(b4 + 1) * 32] = 1.0

    pa = pair_act[0]                                     # [N, N, Cz]
    xa, sa = single_act[0], single_cond[0]               # [N, C_IN]

    in_maps = []
    for c in range(NCORES):
        q0 = c * RQ
        m = dict(shared)
        k0 = q0 - WL
        xkp = np.zeros((RK, C_IN), np.float32)
        skp = np.zeros((RK, C_IN), np.float32)
        lo, hi = max(k0, 0), min(k0 + RK, N)
        xkp[lo - k0:hi - k0] = xa[lo:hi]
        skp[lo - k0:hi - k0] = sa[lo:hi]
        m["xs"] = _bf(np.concatenate(
            [xa[q0:q0 + RQ].T, sa[q0:q0 + RQ].T, xkp.T, skp.T], axis=1))

        zt = np.zeros((128, QB * W), np.float32)
        edge = np.zeros((4, 2, W), np.float32)
        for b in range(NB):
            B = c * NB + b
            js = B * QB - WL + np.arange(W)
            valid = (js >= 0) & (js < N)
            jc = np.clip(js, 0, N - 1)
            band = pa[B * QB:(B + 1) * QB][:, jc, :] * valid[None, :, None]
            # [32, W, 16] -> zt[b*16+cz, i*W+j]
            zt[b * 16:(b + 1) * 16, :] = band.transpose(2, 0, 1).reshape(C_Z, QB * W)
            ok = valid & block_mask[B * QB, jc]
            edge[b % 4, b // 4, :] = np.where(ok, 0.0, NEG)
        m["zt"] = _bf(zt)
        m["small4"] = _bf(np.concatenate(
            [e4, edge.reshape(4, 2 * W)], axis=1))
        in_maps.append(m)

    global _last_in_maps
    _last_in_maps = in_maps
    res = run_bass_kernel_spmd(_built(), in_maps, list(range(NCORES)))
    rows = [np.asarray(res.results[i]["out"], np.float32).T for i in range(NCORES)]
    return np.concatenate(rows, 0).reshape(1, N, C_IN)

